# revision 7
# baseline (speedup 1.0000x reference)
"""CTC-style loss (nn_CTCFormal) on 8 Trainium2 NeuronCores.

Pure data parallel over batch N=4096 -> 512 samples/core, laid out as
[P=128 partitions, G=4 groups].  Groups 0-2 run their alpha recurrences on
the Vector engine (DVE); group 3 runs independently on GpSimd (Pool) so the
two engines stream their own serial chains in parallel with no cross deps.

Formulation: the alpha recurrence is rescaled by the per-step blank
probability.  With a~[t,s] = alpha[t,s] / prod_{tau<=t} y_blank(tau):

  even s (blanks):  a~[t,s] = a~[t-1,s] + a~[t-1,s-1]           (no multiply)
  odd  s (labels):  a~[t,s] = (a~[t-1,s] + a~[t-1,s-1]) * r[t,j]
                              + a~[t-1,s-2] * r2[t,j]
  loss = -( log(a~[T-1,S-1] + a~[T-1,S-2]) + sum_t x_blank[t] )

where r = exp(x_lab - x_blank) and r2 = z * r with z the CTC skip mask
(z=0 when lab[j]==lab[j-1]); r2 comes from exp() of a -1e30-masked copy.
The blank product becomes a log-space reduce_sum of raw blank logits, so
the blank multiply disappears from the inner loop (4 ops/step/engine).

The alpha DP is banded: at step t only states s in [max(0,2t-66),
min(62,2t+1)] are live (states outside cannot lie on any path from
(0,{0,1}) to (T-1,{S-1,S-2})), so ops slice just the live band (~53% of
the elements, exact).

a~ reaches ~1e22; the ACT Ln table is only accurate on ~[1e-15, 1e15], so
the final log runs with scale=2^-32 inside the activation and the 32*ln2
correction is folded into the last tensor_scalar.

Host prep replicates the reference's (buggy) target padding, gathers the
per-sample label-class logit rows (index-only data movement; this
environment's SWDGE gather ucode faults), subtracts the blank row, and
ships bf16 [P, T, G, L] time-major tensors so each step's r slice is
contiguous.  Chunked DMAs + ACT exp (small chunks first, to lead the
early, narrow-band steps) overlap the recurrence.  The host sums the
8x512 partials (the all-reduce of the scalar loss sum).
"""

import numpy as np

T, N, C = 64, 4096, 128
L = 31           # labels per sample
S = 2 * L + 1    # 63 padded states
NCORES = 8
NLOC = N // NCORES          # 512 samples per core
G = NLOC // 128             # 4 groups of 128 samples (partition dim)
GD = G - 1                  # groups on DVE; last group runs on GpSimd
P = 128
CHUNKS = [4, 4, 4, 4, 8, 8, 8, 8, 8, 8]   # T chunking for DMA/exp pipeline

_BASS_CACHE = {}


def _band(t):
    """Live CTC band [lo, hi] (inclusive states) at step t; lo forced even."""
    lo = max(0, 2 * t - 66)
    hi = min(S - 1, 2 * t + 1)
    return lo, hi


def _build_bass():
    if "nc" in _BASS_CACHE:
        return _BASS_CACHE["nc"]

    import concourse.bacc as bacc
    import concourse.mybir as mybir
    from concourse.tile import TileContext

    f32 = mybir.dt.float32
    bf16 = mybir.dt.bfloat16
    AF = mybir.ActivationFunctionType
    Alu = mybir.AluOpType

    nc = bacc.Bacc(trn_type="TRN2")
    xd_d = nc.declare_dram_parameter("xd", [P, T, G, L], bf16, isOutput=False)
    xdm_d = nc.declare_dram_parameter("xdm", [P, T, G, L], bf16, isOutput=False)
    blkl_d = nc.declare_dram_parameter("blkl", [P, G, T], f32, isOutput=False)
    loss_d = nc.declare_dram_parameter("loss", [P, G], f32, isOutput=True)

    with TileContext(nc) as tc:
        with tc.tile_pool(name="main", bufs=1) as pool:
            blkl = pool.tile([P, G, T], f32)
            nc.sync.dma_start(out=blkl[:], in_=blkl_d[:])

            # alpha states (cols 0,1 zero; state s in col s+2)
            aD = pool.tile([P, GD, S + 2], f32)
            bD = pool.tile([P, GD, S + 2], f32)
            vvD = pool.tile([P, GD, L], f32)
            nc.vector.memset(aD[:], 0.0)
            nc.vector.memset(bD[:], 0.0)
            nc.vector.memset(aD[:, :, 2], 1.0)
            aP = pool.tile([P, S + 2], f32)
            bP = pool.tile([P, S + 2], f32)
            vvP = pool.tile([P, L], f32)
            nc.gpsimd.memset(aP[:], 0.0)
            nc.gpsimd.memset(bP[:], 0.0)
            nc.gpsimd.memset(aP[:, 2:3], 1.0)

            xd_s = pool.tile([P, T, G, L], bf16)
            xdm_s = pool.tile([P, T, G, L], bf16)
            r = pool.tile([P, T, G, L], f32)
            r2 = pool.tile([P, T, G, L], f32)
            off = 0
            for ci, ch in enumerate(CHUNKS):
                sl = slice(off, off + ch)
                off += ch
                nc.sync.dma_start(out=xd_s[:, sl], in_=xd_d[:, sl])
                nc.scalar.activation(out=r[:, sl], in_=xd_s[:, sl], func=AF.Exp)
                nc.sync.dma_start(out=xdm_s[:, sl], in_=xdm_d[:, sl])
                nc.scalar.activation(out=r2[:, sl], in_=xdm_s[:, sl], func=AF.Exp)
                if ci == 0:
                    # a~0[s=1] = r[t=0, j=0]; emitted here so ACT runs it
                    # before the later chunks' exps (ACT executes in order)
                    nc.scalar.copy(out=aD[:, :, 3], in_=r[:, 0, :GD, 0])
                    nc.scalar.copy(out=aP[:, 3:4], in_=r[:, 0, GD, 0:1])

            cD, nD = aD, bD
            cP, nP = aP, bP
            for t in range(1, T):
                lo, hi = _band(t)
                clo, chi = lo + 2, hi + 2
                ho = hi if hi % 2 == 1 else hi - 1   # top odd state
                jlo, jhi = lo // 2, (ho - 1) // 2    # inclusive label idx range
                # DVE: groups 0..GD-1
                nc.vector.tensor_add(
                    out=nD[:, :, clo : chi + 1],
                    in0=cD[:, :, clo : chi + 1],
                    in1=cD[:, :, clo - 1 : chi],
                )
                nc.vector.tensor_mul(
                    out=vvD[:, :, jlo : jhi + 1],
                    in0=cD[:, :, 2 * jlo + 1 : 2 * jhi + 2 : 2],
                    in1=r2[:, t, :GD, jlo : jhi + 1],
                )
                nc.vector.tensor_mul(
                    out=nD[:, :, 2 * jlo + 3 : 2 * jhi + 4 : 2],
                    in0=nD[:, :, 2 * jlo + 3 : 2 * jhi + 4 : 2],
                    in1=r[:, t, :GD, jlo : jhi + 1],
                )
                nc.vector.tensor_add(
                    out=nD[:, :, 2 * jlo + 3 : 2 * jhi + 4 : 2],
                    in0=nD[:, :, 2 * jlo + 3 : 2 * jhi + 4 : 2],
                    in1=vvD[:, :, jlo : jhi + 1],
                )
                # GpSimd: group GD
                nc.gpsimd.tensor_add(
                    out=nP[:, clo : chi + 1],
                    in0=cP[:, clo : chi + 1],
                    in1=cP[:, clo - 1 : chi],
                )
                nc.gpsimd.tensor_mul(
                    out=vvP[:, jlo : jhi + 1],
                    in0=cP[:, 2 * jlo + 1 : 2 * jhi + 2 : 2],
                    in1=r2[:, t, GD, jlo : jhi + 1],
                )
                nc.gpsimd.tensor_mul(
                    out=nP[:, 2 * jlo + 3 : 2 * jhi + 4 : 2],
                    in0=nP[:, 2 * jlo + 3 : 2 * jhi + 4 : 2],
                    in1=r[:, t, GD, jlo : jhi + 1],
                )
                nc.gpsimd.tensor_add(
                    out=nP[:, 2 * jlo + 3 : 2 * jhi + 4 : 2],
                    in0=nP[:, 2 * jlo + 3 : 2 * jhi + 4 : 2],
                    in1=vvP[:, jlo : jhi + 1],
                )
                cD, nD = nD, cD
                cP, nP = nP, cP

            # loss = -( log((a~[S-1]+a~[S-2]) * 2^-32) + 32*ln2 + sum_t x_blank )
            stot = pool.tile([P, G], f32)
            nc.vector.tensor_add(
                out=stot[:, :GD], in0=cD[:, :, S + 1], in1=cD[:, :, S]
            )
            nc.vector.tensor_add(
                out=stot[:, GD : GD + 1], in0=cP[:, S + 1 : S + 2], in1=cP[:, S : S + 1]
            )
            lg = pool.tile([P, G], f32)
            nc.scalar.activation(
                out=lg[:], in_=stot[:], func=AF.Ln, scale=float(2.0**-32)
            )
            bsum = pool.tile([P, G, 1], f32)
            nc.vector.reduce_sum(out=bsum[:], in_=blkl[:], axis=mybir.AxisListType.X)
            tot = pool.tile([P, G], f32)
            nc.vector.tensor_add(out=tot[:], in0=lg[:], in1=bsum[:, :, 0])
            neg = pool.tile([P, G], f32)
            nc.vector.tensor_scalar(
                out=neg[:],
                in0=tot[:],
                scalar1=-1.0,
                scalar2=float(-32.0 * np.log(2.0)),
                op0=Alu.mult,
                op1=Alu.add,
            )
            nc.sync.dma_start(out=loss_d[:], in_=neg[:])

    nc.finalize()
    _BASS_CACHE["nc"] = nc
    return nc


def host_prep(input, target, input_length, target_length):
    """Build the 8 per-core input maps."""
    import ml_dtypes

    inp = np.asarray(input, dtype=np.float32)
    target = np.asarray(target, dtype=np.int32)
    tl = np.asarray(target_length, dtype=np.int64)

    # reference's buggy padding: start_i = target_length[i-1] if i>0 else 0,
    # clamped like jax.lax.dynamic_slice
    starts = np.zeros(N, np.int64)
    starts[1:] = tl[: N - 1]
    starts = np.clip(starts, 0, len(target) - L)
    lab = target[starts[:, None] + np.arange(L)]  # [N, L] int32
    z = np.ones((N, L), np.float32)
    z[:, 1:] = (lab[:, 1:] != lab[:, :-1]).astype(np.float32)

    x_nct = inp.transpose(1, 2, 0)  # [N, C, T] view
    xs = np.take_along_axis(x_nct, lab[:, :, None].astype(np.int64), axis=1)
    blk = x_nct[:, 0, :]                       # [N, T]
    xd = xs - blk[:, None, :]                  # [N, L, T]
    xdm = np.where(z[:, :, None] == 0.0, np.float32(-1e30), xd)
    xd = np.ascontiguousarray(xd.transpose(0, 2, 1)).astype(ml_dtypes.bfloat16)
    xdm = np.ascontiguousarray(xdm.transpose(0, 2, 1)).astype(ml_dtypes.bfloat16)

    in_maps = []
    for core in range(NCORES):
        sl = slice(core * NLOC, (core + 1) * NLOC)
        xd_c = xd[sl].reshape(G, P, T, L).transpose(1, 2, 0, 3)
        xdm_c = xdm[sl].reshape(G, P, T, L).transpose(1, 2, 0, 3)
        blk_c = blk[sl].reshape(G, P, T).transpose(1, 0, 2)
        in_maps.append(
            {
                "xd": np.ascontiguousarray(xd_c),
                "xdm": np.ascontiguousarray(xdm_c),
                "blkl": np.ascontiguousarray(blk_c),
            }
        )
    return in_maps


def kernel(input, target, input_length, target_length):
    from concourse.bass_utils import run_bass_kernel_spmd

    nc = _build_bass()
    in_maps = host_prep(input, target, input_length, target_length)
    res = run_bass_kernel_spmd(nc, in_maps, list(range(NCORES)))
    total = 0.0
    for core in range(NCORES):
        total += float(np.asarray(res.results[core]["loss"], dtype=np.float64).sum())
    return np.float32(total)


# revision 8
# speedup vs baseline: 1.3187x; 1.3187x over previous
"""CTC-style loss (nn_CTCFormal) on 8 Trainium2 NeuronCores.

Pure data parallel over batch N=4096 -> 512 samples/core, laid out as
[P=128 partitions, G=4 groups] with the whole alpha recurrence on the
Vector engine (a GpSimd group-offload was tried and measured slower: Pool
TT ops have ~230 ns fixed cost vs DVE's ~50 ns marginal per group).

Formulation: the alpha recurrence is rescaled by the per-step blank
probability.  With a~[t,s] = alpha[t,s] / prod_{tau<=t} y_blank(tau):

  even s (blanks):  a~[t,s] = a~[t-1,s] + a~[t-1,s-1]           (no multiply)
  odd  s (labels):  a~[t,s] = (a~[t-1,s] + a~[t-1,s-1]
                               + a~[t-1,s-2] * z[j]) * r[t,j]
  loss = -( log(a~[T-1,S-1] + a~[T-1,S-2]) + sum_t x_blank[t] )

where r = exp(x_lab - x_blank) and z is the static CTC skip mask (0 when
lab[j]==lab[j-1]).  The blank product becomes a log-space reduce_sum of
raw blank logits, so the inner loop is 4 DVE ops/step and only the final
multiply reads ACT-produced data (one cross-engine wait per step).

The alpha DP is banded: at step t only states s in [max(0,2t-66),
min(62,2t+1)] are live (states outside cannot lie on any path from
(0,{0,1}) to (T-1,{S-1,S-2})), so ops slice just the live band (~53% of
the elements, exact).

a~ reaches ~1e22; the ACT Ln table is only accurate on ~[1e-15, 1e15], so
the final log runs with scale=2^-32 inside the activation and the 32*ln2
correction is folded into the last tensor_scalar.

Host prep replicates the reference's (buggy) target padding, gathers the
per-sample label-class logit rows (index-only data movement; this
environment's SWDGE gather ucode faults), subtracts the blank row, and
ships a bf16 [P, T, G, L] time-major tensor so each step's r slice is
contiguous.  Chunked DMAs + ACT exp (small chunks first, to lead the
early narrow-band steps) overlap the recurrence.  The host sums the
8x512 partials (the all-reduce of the scalar loss sum).
"""

import numpy as np

T, N, C = 64, 4096, 128
L = 31           # labels per sample
S = 2 * L + 1    # 63 padded states
NCORES = 8
NLOC = N // NCORES          # 512 samples per core
G = NLOC // 128             # 4 groups of 128 samples (partition dim)
P = 128
CHUNKS = [4, 4, 4, 4, 8, 8, 8, 8, 8, 8]   # T chunking for DMA/exp pipeline

_BASS_CACHE = {}


def _band(t):
    """Live CTC band [lo, hi] (inclusive states) at step t; lo forced even."""
    lo = max(0, 2 * t - 66)
    hi = min(S - 1, 2 * t + 1)
    return lo, hi


def _build_bass():
    if "nc" in _BASS_CACHE:
        return _BASS_CACHE["nc"]

    import concourse.bacc as bacc
    import concourse.mybir as mybir
    from concourse.tile import TileContext

    f32 = mybir.dt.float32
    bf16 = mybir.dt.bfloat16
    AF = mybir.ActivationFunctionType
    Alu = mybir.AluOpType

    nc = bacc.Bacc(trn_type="TRN2")
    xd_d = nc.declare_dram_parameter("xd", [P, T, G, L], bf16, isOutput=False)
    z_d = nc.declare_dram_parameter("z", [P, G, L], f32, isOutput=False)
    blkl_d = nc.declare_dram_parameter("blkl", [P, G, T], f32, isOutput=False)
    loss_d = nc.declare_dram_parameter("loss", [P, G], f32, isOutput=True)

    with TileContext(nc) as tc:
        with tc.tile_pool(name="main", bufs=1) as pool:
            blkl = pool.tile([P, G, T], f32)
            nc.sync.dma_start(out=blkl[:], in_=blkl_d[:])
            z_up = pool.tile([P, G, L], f32)
            nc.sync.dma_start(out=z_up[:], in_=z_d[:])
            # DVE-owned copy: in-loop reads then carry no DMA wait (each
            # instruction has a single HW wait slot; a second wait costs an
            # event-semaphore instruction per step)
            z = pool.tile([P, G, L], f32)
            nc.vector.tensor_copy(out=z[:], in_=z_up[:])

            # alpha states (cols 0,1 zero; state s in col s+2)
            a = pool.tile([P, G, S + 2], f32)
            b = pool.tile([P, G, S + 2], f32)
            vv = pool.tile([P, G, L], f32)
            nc.vector.memset(a[:], 0.0)
            nc.vector.memset(b[:], 0.0)
            nc.vector.memset(a[:, :, 2], 1.0)

            xd_s = pool.tile([P, T, G, L], bf16)
            r = pool.tile([P, T, G, L], f32)
            off = 0
            for ci, ch in enumerate(CHUNKS):
                sl = slice(off, off + ch)
                off += ch
                nc.sync.dma_start(out=xd_s[:, sl], in_=xd_d[:, sl])
                nc.scalar.activation(out=r[:, sl], in_=xd_s[:, sl], func=AF.Exp)
                if ci == 0:
                    # a~0[s=1] = r[t=0, j=0]; emitted here so ACT runs it
                    # before the later chunks' exps (ACT executes in order)
                    nc.scalar.copy(out=a[:, :, 3], in_=r[:, 0, :, 0])

            cur, nxt = a, b
            for t in range(1, T):
                lo, hi = _band(t)
                clo, chi = lo + 2, hi + 2
                ho = hi if hi % 2 == 1 else hi - 1   # top odd state
                jlo, jhi = lo // 2, (ho - 1) // 2    # inclusive label idx range
                # vv[j] = a~[t-1, 2j-1] * z[j]   (cur col 2j+1)
                nc.vector.tensor_mul(
                    out=vv[:, :, jlo : jhi + 1],
                    in0=cur[:, :, 2 * jlo + 1 : 2 * jhi + 2 : 2],
                    in1=z[:, :, jlo : jhi + 1],
                )
                # nxt[s] = cur[s] + cur[s-1] over the band (both parities)
                nc.vector.tensor_add(
                    out=nxt[:, :, clo : chi + 1],
                    in0=cur[:, :, clo : chi + 1],
                    in1=cur[:, :, clo - 1 : chi],
                )
                # odd lanes: += vv, then *= r
                nc.vector.tensor_add(
                    out=nxt[:, :, 2 * jlo + 3 : 2 * jhi + 4 : 2],
                    in0=nxt[:, :, 2 * jlo + 3 : 2 * jhi + 4 : 2],
                    in1=vv[:, :, jlo : jhi + 1],
                )
                nc.vector.tensor_mul(
                    out=nxt[:, :, 2 * jlo + 3 : 2 * jhi + 4 : 2],
                    in0=nxt[:, :, 2 * jlo + 3 : 2 * jhi + 4 : 2],
                    in1=r[:, t, :, jlo : jhi + 1],
                )
                cur, nxt = nxt, cur

            # loss = -( log((a~[S-1]+a~[S-2]) * 2^-32) + 32*ln2 + sum_t x_blank )
            stot = pool.tile([P, G], f32)
            nc.vector.tensor_add(
                out=stot[:], in0=cur[:, :, S + 1], in1=cur[:, :, S]
            )
            lg = pool.tile([P, G], f32)
            nc.scalar.activation(
                out=lg[:], in_=stot[:], func=AF.Ln, scale=float(2.0**-32)
            )
            bsum = pool.tile([P, G, 1], f32)
            nc.vector.reduce_sum(out=bsum[:], in_=blkl[:], axis=mybir.AxisListType.X)
            tot = pool.tile([P, G], f32)
            nc.vector.tensor_add(out=tot[:], in0=lg[:], in1=bsum[:, :, 0])
            neg = pool.tile([P, G], f32)
            nc.vector.tensor_scalar(
                out=neg[:],
                in0=tot[:],
                scalar1=-1.0,
                scalar2=float(-32.0 * np.log(2.0)),
                op0=Alu.mult,
                op1=Alu.add,
            )
            nc.sync.dma_start(out=loss_d[:], in_=neg[:])

    nc.finalize()
    _BASS_CACHE["nc"] = nc
    return nc


def host_prep(input, target, input_length, target_length):
    """Build the 8 per-core input maps."""
    import ml_dtypes

    inp = np.asarray(input, dtype=np.float32)
    target = np.asarray(target, dtype=np.int32)
    tl = np.asarray(target_length, dtype=np.int64)

    # reference's buggy padding: start_i = target_length[i-1] if i>0 else 0,
    # clamped like jax.lax.dynamic_slice
    starts = np.zeros(N, np.int64)
    starts[1:] = tl[: N - 1]
    starts = np.clip(starts, 0, len(target) - L)
    lab = target[starts[:, None] + np.arange(L)]  # [N, L] int32
    z = np.ones((N, L), np.float32)
    z[:, 1:] = (lab[:, 1:] != lab[:, :-1]).astype(np.float32)

    x_nct = inp.transpose(1, 2, 0)  # [N, C, T] view
    xs = np.take_along_axis(x_nct, lab[:, :, None].astype(np.int64), axis=1)
    blk = x_nct[:, 0, :]                       # [N, T]
    xd = xs - blk[:, None, :]                  # [N, L, T]
    xd = np.ascontiguousarray(xd.transpose(0, 2, 1)).astype(ml_dtypes.bfloat16)

    in_maps = []
    for core in range(NCORES):
        sl = slice(core * NLOC, (core + 1) * NLOC)
        xd_c = xd[sl].reshape(G, P, T, L).transpose(1, 2, 0, 3)
        z_c = z[sl].reshape(G, P, L).transpose(1, 0, 2)
        blk_c = blk[sl].reshape(G, P, T).transpose(1, 0, 2)
        in_maps.append(
            {
                "xd": np.ascontiguousarray(xd_c),
                "z": np.ascontiguousarray(z_c),
                "blkl": np.ascontiguousarray(blk_c),
            }
        )
    return in_maps


def kernel(input, target, input_length, target_length):
    from concourse.bass_utils import run_bass_kernel_spmd

    nc = _build_bass()
    in_maps = host_prep(input, target, input_length, target_length)
    res = run_bass_kernel_spmd(nc, in_maps, list(range(NCORES)))
    total = 0.0
    for core in range(NCORES):
        total += float(np.asarray(res.results[core]["loss"], dtype=np.float64).sum())
    return np.float32(total)


# revision 9
# speedup vs baseline: 1.4901x; 1.1299x over previous
"""CTC-style loss (nn_CTCFormal) on 8 Trainium2 NeuronCores.

Pure data parallel over batch N=4096 -> 512 samples/core, laid out as
[P=128 partitions, G=4 groups] with the whole alpha recurrence on the
Vector engine (a GpSimd group-offload was tried and measured slower: Pool
TT ops have ~230 ns fixed cost vs DVE's ~50 ns marginal per group).

Formulation: the alpha recurrence is rescaled by the per-step blank
probability.  With a~[t,s] = alpha[t,s] / prod_{tau<=t} y_blank(tau):

  even s (blanks):  a~[t,s] = a~[t-1,s] + a~[t-1,s-1]           (no multiply)
  odd  s (labels):  a~[t,s] = (a~[t-1,s] + a~[t-1,s-1]
                               + a~[t-1,s-2] * z[j]) * r[t,j]
  loss = -( log(a~[T-1,S-1] + a~[T-1,S-2]) + sum_t x_blank[t] )

where r = exp(x_lab - x_blank) and z is the static CTC skip mask (0 when
lab[j]==lab[j-1]).  The blank product becomes a log-space reduce_sum of
raw blank logits, so the inner loop is 4 DVE ops/step and only the final
multiply reads ACT-produced data (one cross-engine wait per step).

The alpha DP is banded: at step t only states s in [max(0,2t-66),
min(62,2t+1)] are live (states outside cannot lie on any path from
(0,{0,1}) to (T-1,{S-1,S-2})), so ops slice just the live band (~53% of
the elements, exact).

a~ reaches ~1e22; the ACT Ln table is only accurate on ~[1e-15, 1e15], so
the final log runs with scale=2^-32 inside the activation and the 32*ln2
correction is folded into the last tensor_scalar.

Host prep replicates the reference's (buggy) target padding, gathers the
per-sample label-class logit rows (index-only data movement; this
environment's SWDGE gather ucode faults), subtracts the blank row, and
ships a bf16 [P, T, G, L] time-major tensor so each step's r slice is
contiguous.  Chunked DMAs + ACT exp (small chunks first, to lead the
early narrow-band steps) overlap the recurrence.  The host sums the
8x512 partials (the all-reduce of the scalar loss sum).
"""

import numpy as np

T, N, C = 64, 4096, 128
L = 31           # labels per sample
S = 2 * L + 1    # 63 padded states
NCORES = 8
NLOC = N // NCORES          # 512 samples per core
G = NLOC // 128             # 4 groups of 128 samples (partition dim)
P = 128
CHUNKS = [4, 4, 4, 4, 8, 8, 8, 8, 8, 8]   # T chunking for DMA/exp pipeline

_BASS_CACHE = {}


def _band(t):
    """Live CTC band [lo, hi] (inclusive states) at step t; lo forced even."""
    lo = max(0, 2 * t - 66)
    hi = min(S - 1, 2 * t + 1)
    return lo, hi


def _build_bass():
    if "nc" in _BASS_CACHE:
        return _BASS_CACHE["nc"]

    import concourse.bacc as bacc
    import concourse.mybir as mybir
    from concourse.tile import TileContext

    f32 = mybir.dt.float32
    bf16 = mybir.dt.bfloat16
    AF = mybir.ActivationFunctionType
    Alu = mybir.AluOpType

    nc = bacc.Bacc(trn_type="TRN2")
    xd_d = nc.declare_dram_parameter("xd", [P, T, G, L], bf16, isOutput=False)
    xdm_d = nc.declare_dram_parameter("xdm", [P, T, G, L], bf16, isOutput=False)
    blkl_d = nc.declare_dram_parameter("blkl", [P, G, T], f32, isOutput=False)
    loss_d = nc.declare_dram_parameter("loss", [P, G], f32, isOutput=True)

    with TileContext(nc) as tc:
        with tc.tile_pool(name="main", bufs=1) as pool:
            blkl = pool.tile([P, G, T], f32)
            nc.sync.dma_start(out=blkl[:], in_=blkl_d[:])
            # alpha states (cols 0,1 zero; state s in col s+2)
            a = pool.tile([P, G, S + 2], f32)
            b = pool.tile([P, G, S + 2], f32)
            vv = pool.tile([P, G, L], f32)
            nc.vector.memset(a[:], 0.0)
            nc.vector.memset(b[:], 0.0)
            nc.vector.memset(a[:, :, 2], 1.0)

            xd_s = pool.tile([P, T, G, L], bf16)
            xdm_s = pool.tile([P, T, G, L], bf16)
            r = pool.tile([P, T, G, L], f32)
            r2 = pool.tile([P, T, G, L], f32)
            off = 0
            for ci, ch in enumerate(CHUNKS):
                sl = slice(off, off + ch)
                off += ch
                nc.sync.dma_start(out=xd_s[:, sl], in_=xd_d[:, sl])
                nc.scalar.activation(out=r[:, sl], in_=xd_s[:, sl], func=AF.Exp)
                nc.sync.dma_start(out=xdm_s[:, sl], in_=xdm_d[:, sl])
                nc.scalar.activation(out=r2[:, sl], in_=xdm_s[:, sl], func=AF.Exp)
                if ci == 0:
                    # a~0[s=1] = r[t=0, j=0]; emitted here so ACT runs it
                    # before the later chunks' exps (ACT executes in order)
                    nc.scalar.copy(out=a[:, :, 3], in_=r[:, 0, :, 0])

            cur, nxt = a, b
            for t in range(1, T):
                lo, hi = _band(t)
                clo, chi = lo + 2, hi + 2
                ho = hi if hi % 2 == 1 else hi - 1   # top odd state
                jlo, jhi = lo // 2, (ho - 1) // 2    # inclusive label idx range
                # nxt[s] = cur[s] + cur[s-1] over the band (both parities)
                nc.vector.tensor_add(
                    out=nxt[:, :, clo : chi + 1],
                    in0=cur[:, :, clo : chi + 1],
                    in1=cur[:, :, clo - 1 : chi],
                )
                # vv[j] = a~[t-1, 2j-1] * r2[t,j]   (cur col 2j+1)
                nc.vector.tensor_mul(
                    out=vv[:, :, jlo : jhi + 1],
                    in0=cur[:, :, 2 * jlo + 1 : 2 * jhi + 2 : 2],
                    in1=r2[:, t, :, jlo : jhi + 1],
                )
                # odd lanes: *= r, then += vv  (this order keeps only one
                # adjacent RAW pair per step; engines have no interlocks, so
                # a dependent op one slot behind pays the write-ack latency)
                nc.vector.tensor_mul(
                    out=nxt[:, :, 2 * jlo + 3 : 2 * jhi + 4 : 2],
                    in0=nxt[:, :, 2 * jlo + 3 : 2 * jhi + 4 : 2],
                    in1=r[:, t, :, jlo : jhi + 1],
                )
                nc.vector.tensor_add(
                    out=nxt[:, :, 2 * jlo + 3 : 2 * jhi + 4 : 2],
                    in0=nxt[:, :, 2 * jlo + 3 : 2 * jhi + 4 : 2],
                    in1=vv[:, :, jlo : jhi + 1],
                )
                cur, nxt = nxt, cur

            # loss = -( log((a~[S-1]+a~[S-2]) * 2^-32) + 32*ln2 + sum_t x_blank )
            stot = pool.tile([P, G], f32)
            nc.vector.tensor_add(
                out=stot[:], in0=cur[:, :, S + 1], in1=cur[:, :, S]
            )
            lg = pool.tile([P, G], f32)
            nc.scalar.activation(
                out=lg[:], in_=stot[:], func=AF.Ln, scale=float(2.0**-32)
            )
            bsum = pool.tile([P, G, 1], f32)
            nc.vector.reduce_sum(out=bsum[:], in_=blkl[:], axis=mybir.AxisListType.X)
            tot = pool.tile([P, G], f32)
            nc.vector.tensor_add(out=tot[:], in0=lg[:], in1=bsum[:, :, 0])
            neg = pool.tile([P, G], f32)
            nc.vector.tensor_scalar(
                out=neg[:],
                in0=tot[:],
                scalar1=-1.0,
                scalar2=float(-32.0 * np.log(2.0)),
                op0=Alu.mult,
                op1=Alu.add,
            )
            nc.sync.dma_start(out=loss_d[:], in_=neg[:])

    nc.finalize()
    _BASS_CACHE["nc"] = nc
    return nc


def host_prep(input, target, input_length, target_length):
    """Build the 8 per-core input maps."""
    import ml_dtypes

    inp = np.asarray(input, dtype=np.float32)
    target = np.asarray(target, dtype=np.int32)
    tl = np.asarray(target_length, dtype=np.int64)

    # reference's buggy padding: start_i = target_length[i-1] if i>0 else 0,
    # clamped like jax.lax.dynamic_slice
    starts = np.zeros(N, np.int64)
    starts[1:] = tl[: N - 1]
    starts = np.clip(starts, 0, len(target) - L)
    lab = target[starts[:, None] + np.arange(L)]  # [N, L] int32
    z = np.ones((N, L), np.float32)
    z[:, 1:] = (lab[:, 1:] != lab[:, :-1]).astype(np.float32)

    x_nct = inp.transpose(1, 2, 0)  # [N, C, T] view
    xs = np.take_along_axis(x_nct, lab[:, :, None].astype(np.int64), axis=1)
    blk = x_nct[:, 0, :]                       # [N, T]
    xd = xs - blk[:, None, :]                  # [N, L, T]
    xdm = np.where(z[:, :, None] == 0.0, np.float32(-1e30), xd)
    xd = np.ascontiguousarray(xd.transpose(0, 2, 1)).astype(ml_dtypes.bfloat16)
    xdm = np.ascontiguousarray(xdm.transpose(0, 2, 1)).astype(ml_dtypes.bfloat16)

    in_maps = []
    for core in range(NCORES):
        sl = slice(core * NLOC, (core + 1) * NLOC)
        xd_c = xd[sl].reshape(G, P, T, L).transpose(1, 2, 0, 3)
        xdm_c = xdm[sl].reshape(G, P, T, L).transpose(1, 2, 0, 3)
        blk_c = blk[sl].reshape(G, P, T).transpose(1, 0, 2)
        in_maps.append(
            {
                "xd": np.ascontiguousarray(xd_c),
                "xdm": np.ascontiguousarray(xdm_c),
                "blkl": np.ascontiguousarray(blk_c),
            }
        )
    return in_maps


def kernel(input, target, input_length, target_length):
    from concourse.bass_utils import run_bass_kernel_spmd

    nc = _build_bass()
    in_maps = host_prep(input, target, input_length, target_length)
    res = run_bass_kernel_spmd(nc, in_maps, list(range(NCORES)))
    total = 0.0
    for core in range(NCORES):
        total += float(np.asarray(res.results[core]["loss"], dtype=np.float64).sum())
    return np.float32(total)


# revision 10
# speedup vs baseline: 1.4991x; 1.0061x over previous
"""CTC-style loss (nn_CTCFormal) on 8 Trainium2 NeuronCores.

Pure data parallel over batch N=4096 -> 512 samples/core, laid out as
[P=128 partitions, G=4 groups] with the whole alpha recurrence on the
Vector engine (a GpSimd group-offload was tried and measured slower: Pool
TT ops have ~230 ns fixed cost vs DVE's ~50 ns marginal per group).

Formulation: the alpha recurrence is rescaled by the per-step blank
probability.  With a~[t,s] = alpha[t,s] / prod_{tau<=t} y_blank(tau):

  even s (blanks):  a~[t,s] = a~[t-1,s] + a~[t-1,s-1]           (no multiply)
  odd  s (labels):  a~[t,s] = (a~[t-1,s] + a~[t-1,s-1]
                               + a~[t-1,s-2] * z[j]) * r[t,j]
  loss = -( log(a~[T-1,S-1] + a~[T-1,S-2]) + sum_t x_blank[t] )

where r = exp(x_lab - x_blank) and z is the static CTC skip mask (0 when
lab[j]==lab[j-1]).  The blank product becomes a log-space reduce_sum of
raw blank logits, so the inner loop is 4 DVE ops/step and only the final
multiply reads ACT-produced data (one cross-engine wait per step).

The alpha DP is banded: at step t only states s in [max(0,2t-66),
min(62,2t+1)] are live (states outside cannot lie on any path from
(0,{0,1}) to (T-1,{S-1,S-2})), so ops slice just the live band (~53% of
the elements, exact).

a~ reaches ~1e22; the ACT Ln table is only accurate on ~[1e-15, 1e15], so
the final log runs with scale=2^-32 inside the activation and the 32*ln2
correction is folded into the last tensor_scalar.

Host prep replicates the reference's (buggy) target padding, gathers the
per-sample label-class logit rows (index-only data movement; this
environment's SWDGE gather ucode faults), subtracts the blank row, and
ships a bf16 [P, T, G, L] time-major tensor so each step's r slice is
contiguous.  Chunked DMAs + ACT exp (small chunks first, to lead the
early narrow-band steps) overlap the recurrence.  The host sums the
8x512 partials (the all-reduce of the scalar loss sum).
"""

import numpy as np

T, N, C = 64, 4096, 128
L = 31           # labels per sample
S = 2 * L + 1    # 63 padded states
NCORES = 8
NLOC = N // NCORES          # 512 samples per core
G = NLOC // 128             # 4 groups of 128 samples (partition dim)
P = 128
CHUNKS = [2, 2, 2, 2, 4, 4, 4, 4, 8, 8, 8, 8, 8]   # T chunking for DMA/exp pipeline

_BASS_CACHE = {}


def _band(t):
    """Live CTC band [lo, hi] (inclusive states) at step t; lo forced even."""
    lo = max(0, 2 * t - 66)
    hi = min(S - 1, 2 * t + 1)
    return lo, hi


def _build_bass():
    if "nc" in _BASS_CACHE:
        return _BASS_CACHE["nc"]

    import concourse.bacc as bacc
    import concourse.mybir as mybir
    from concourse.tile import TileContext

    f32 = mybir.dt.float32
    bf16 = mybir.dt.bfloat16
    AF = mybir.ActivationFunctionType
    Alu = mybir.AluOpType

    nc = bacc.Bacc(trn_type="TRN2")
    xd_d = nc.declare_dram_parameter("xd", [P, T, G, L], bf16, isOutput=False)
    xdm_d = nc.declare_dram_parameter("xdm", [P, T, G, L], bf16, isOutput=False)
    blkl_d = nc.declare_dram_parameter("blkl", [P, G, T], f32, isOutput=False)
    loss_d = nc.declare_dram_parameter("loss", [P, G], f32, isOutput=True)

    with TileContext(nc) as tc:
        with tc.tile_pool(name="main", bufs=1) as pool:
            blkl = pool.tile([P, G, T], f32)
            nc.sync.dma_start(out=blkl[:], in_=blkl_d[:])
            # alpha states (cols 0,1 zero; state s in col s+2)
            a = pool.tile([P, G, S + 2], f32)
            b = pool.tile([P, G, S + 2], f32)
            vv = pool.tile([P, G, L], f32)
            nc.vector.memset(a[:], 0.0)
            nc.vector.memset(b[:], 0.0)
            nc.vector.memset(a[:, :, 2], 1.0)

            xd_s = pool.tile([P, T, G, L], bf16)
            xdm_s = pool.tile([P, T, G, L], bf16)
            r = pool.tile([P, T, G, L], f32)
            r2 = pool.tile([P, T, G, L], f32)
            off = 0
            for ci, ch in enumerate(CHUNKS):
                sl = slice(off, off + ch)
                off += ch
                nc.sync.dma_start(out=xd_s[:, sl], in_=xd_d[:, sl])
                nc.scalar.activation(out=r[:, sl], in_=xd_s[:, sl], func=AF.Exp)
                nc.sync.dma_start(out=xdm_s[:, sl], in_=xdm_d[:, sl])
                nc.scalar.activation(out=r2[:, sl], in_=xdm_s[:, sl], func=AF.Exp)
                if ci == 0:
                    # a~0[s=1] = r[t=0, j=0]; emitted here so ACT runs it
                    # before the later chunks' exps (ACT executes in order)
                    nc.scalar.copy(out=a[:, :, 3], in_=r[:, 0, :, 0])

            cur, nxt = a, b
            for t in range(1, T):
                lo, hi = _band(t)
                clo, chi = lo + 2, hi + 2
                ho = hi if hi % 2 == 1 else hi - 1   # top odd state
                jlo, jhi = lo // 2, (ho - 1) // 2    # inclusive label idx range
                # nxt[s] = cur[s] + cur[s-1] over the band (both parities)
                nc.vector.tensor_add(
                    out=nxt[:, :, clo : chi + 1],
                    in0=cur[:, :, clo : chi + 1],
                    in1=cur[:, :, clo - 1 : chi],
                )
                # vv[j] = a~[t-1, 2j-1] * r2[t,j]   (cur col 2j+1)
                nc.vector.tensor_mul(
                    out=vv[:, :, jlo : jhi + 1],
                    in0=cur[:, :, 2 * jlo + 1 : 2 * jhi + 2 : 2],
                    in1=r2[:, t, :, jlo : jhi + 1],
                )
                # odd lanes: *= r, then += vv  (this order keeps only one
                # adjacent RAW pair per step; engines have no interlocks, so
                # a dependent op one slot behind pays the write-ack latency)
                nc.vector.tensor_mul(
                    out=nxt[:, :, 2 * jlo + 3 : 2 * jhi + 4 : 2],
                    in0=nxt[:, :, 2 * jlo + 3 : 2 * jhi + 4 : 2],
                    in1=r[:, t, :, jlo : jhi + 1],
                )
                nc.vector.tensor_add(
                    out=nxt[:, :, 2 * jlo + 3 : 2 * jhi + 4 : 2],
                    in0=nxt[:, :, 2 * jlo + 3 : 2 * jhi + 4 : 2],
                    in1=vv[:, :, jlo : jhi + 1],
                )
                cur, nxt = nxt, cur

            # loss = -( log((a~[S-1]+a~[S-2]) * 2^-32) + 32*ln2 + sum_t x_blank )
            stot = pool.tile([P, G], f32)
            nc.vector.tensor_add(
                out=stot[:], in0=cur[:, :, S + 1], in1=cur[:, :, S]
            )
            lg = pool.tile([P, G], f32)
            nc.scalar.activation(
                out=lg[:], in_=stot[:], func=AF.Ln, scale=float(2.0**-32)
            )
            # blkl is uploaded with +32*ln2/T folded into every element, so
            # bsum already carries the Ln pre-scale correction
            bsum = pool.tile([P, G, 1], f32)
            nc.vector.reduce_sum(out=bsum[:], in_=blkl[:], axis=mybir.AxisListType.X)
            neg = pool.tile([P, G], f32)
            nc.vector.scalar_tensor_tensor(
                out=neg[:],
                in0=lg[:],
                scalar=-1.0,
                in1=bsum[:, :, 0],
                op0=Alu.mult,
                op1=Alu.subtract,
            )
            nc.sync.dma_start(out=loss_d[:], in_=neg[:])

    nc.finalize()
    _BASS_CACHE["nc"] = nc
    return nc


def host_prep(input, target, input_length, target_length):
    """Build the 8 per-core input maps."""
    import ml_dtypes

    inp = np.asarray(input, dtype=np.float32)
    target = np.asarray(target, dtype=np.int32)
    tl = np.asarray(target_length, dtype=np.int64)

    # reference's buggy padding: start_i = target_length[i-1] if i>0 else 0,
    # clamped like jax.lax.dynamic_slice
    starts = np.zeros(N, np.int64)
    starts[1:] = tl[: N - 1]
    starts = np.clip(starts, 0, len(target) - L)
    lab = target[starts[:, None] + np.arange(L)]  # [N, L] int32
    z = np.ones((N, L), np.float32)
    z[:, 1:] = (lab[:, 1:] != lab[:, :-1]).astype(np.float32)

    x_nct = inp.transpose(1, 2, 0)  # [N, C, T] view
    xs = np.take_along_axis(x_nct, lab[:, :, None].astype(np.int64), axis=1)
    blk = x_nct[:, 0, :]                       # [N, T]
    xd = xs - blk[:, None, :]                  # [N, L, T]
    xdm = np.where(z[:, :, None] == 0.0, np.float32(-1e30), xd)
    xd = np.ascontiguousarray(xd.transpose(0, 2, 1)).astype(ml_dtypes.bfloat16)
    xdm = np.ascontiguousarray(xdm.transpose(0, 2, 1)).astype(ml_dtypes.bfloat16)

    in_maps = []
    for core in range(NCORES):
        sl = slice(core * NLOC, (core + 1) * NLOC)
        xd_c = xd[sl].reshape(G, P, T, L).transpose(1, 2, 0, 3)
        xdm_c = xdm[sl].reshape(G, P, T, L).transpose(1, 2, 0, 3)
        blk_c = (blk[sl] + np.float32(32.0 * np.log(2.0) / T)).reshape(G, P, T).transpose(
            1, 0, 2
        )
        in_maps.append(
            {
                "xd": np.ascontiguousarray(xd_c),
                "xdm": np.ascontiguousarray(xdm_c),
                "blkl": np.ascontiguousarray(blk_c),
            }
        )
    return in_maps


def kernel(input, target, input_length, target_length):
    from concourse.bass_utils import run_bass_kernel_spmd

    nc = _build_bass()
    in_maps = host_prep(input, target, input_length, target_length)
    res = run_bass_kernel_spmd(nc, in_maps, list(range(NCORES)))
    total = 0.0
    for core in range(NCORES):
        total += float(np.asarray(res.results[core]["loss"], dtype=np.float64).sum())
    return np.float32(total)
